# revision 85
# baseline (speedup 1.0000x reference)
"""T5-style encoder layer (pre-LN, RMSNorm, relative-position bias) on 8 trn2
NeuronCores, data-parallel over the batch dimension (B=8 -> one batch element
per core). Each core runs the full layer for its [S, D] slice.

The four linear GEMMs (QKV, attn-out, MLP in/out) run in fp8e4 DoubleRow with
full error compensation: y = Qw.Qa + Qdw.Qa + Qw.Qda, where Qd* are the fp8
quantization residuals. All descale constants fold into existing scale args.
Attention core (logits / exp / bias-mult / attn@v) stays fp32r/bf16.

Self-contained: hardcodes all shapes; only depends on /opt/trn_rl_repo.
"""

import sys

if "/opt/trn_rl_repo" not in sys.path:
    sys.path.insert(0, "/opt/trn_rl_repo")

import numpy as np
import ml_dtypes

import concourse.bass as bass
import concourse.tile as tile
from concourse import bacc
from concourse import mybir
from concourse.masks import make_identity

# ---- problem constants -----------------------------------------------------
B, S, D = 8, 1024, 1024
H, HD = 16, 64
MLP = 4096
NUM_BUCKETS, MAX_DIST = 32, 128
EPS = 1e-6
NCORES = 8
P = 128
NS = S // P        # 8 token tiles
ND = D // P        # 8 feature tiles
NM = MLP // P      # 32 mlp tiles
NDIAG = 2 * NS - 1  # 15 distinct 128x128 tile-diagonals of the bias

F32 = mybir.dt.float32
F32R = mybir.dt.float32r
BF16 = mybir.dt.bfloat16
FP8 = mybir.dt.float8e4
BF16NP = ml_dtypes.bfloat16
FP8NP = ml_dtypes.float8_e4m3

# fp8 scale constants (power of two; absorbed into activation scale args).
# Activations quantize at natural scale (e4m3 is floating point -- scale only
# matters for range/subnormal floors, all values here sit comfortably inside).
SH = 1.0           # h / h2 (rmsnorm outputs, |.| <= ~6)
SW = 32.0          # wq/wk/wv/wo (sd 1/32 -> unit rms)
SWI = 16.0         # wi
SWM = 64.0         # wo_mlp (sd 1/64 -> unit rms)
A_QKV = SH * SW    # scale of q/k/v psum values (32)
SA = 1.0           # attn (post-softmax output, |.| <= ~6)
ONES_VAL = A_QKV / SA          # v_ext ones column: makes attnT = SA * attn
B_WO = 1.0 / (SA * SW)         # descale for attn @ wo psum
# wi psum = (SH*SWI) * a = 16a; yT = relu(psum) directly = 16*relu(a), so the
# relu residual is a single (max,subtract) op with no scale juggling.
SY = SH * SWI      # 16
B_WM = 1.0 / (SY * SWM)        # descale for y @ womlp psum
EXPSC = 1.0 / (A_QKV * A_QKV)  # exp reads logits psum scaled by A_QKV^2


# ---- host-side relative position bias --------------------------------------
def _rel_pos_bucket_np(rel):
    n = -rel
    num_buckets = NUM_BUCKETS // 2          # 16
    ret = (n < 0).astype(np.int32) * num_buckets
    n = np.abs(n)
    max_exact = num_buckets // 2            # 8
    is_small = n < max_exact
    val_if_large = max_exact + (
        np.log(n.astype(np.float32) / max_exact + np.finfo(np.float32).eps)
        / np.log(MAX_DIST / max_exact)
        * (num_buckets - max_exact)
    ).astype(np.int32)
    val_if_large = np.minimum(val_if_large, num_buckets - 1)
    return ret + np.where(is_small, n, val_if_large)


def _bias_blocks(rel_emb):
    """[H, 128, NDIAG, 128] f32 blocks of the transposed bias.

    Block d' (=7-m, m = k_tile - q_tile) at [p, c] = bias^T[k, q] for
    k = k_tile*128 + p, q = q_tile*128 + c, i.e. table[1023 + m*128 + p - c].
    """
    rel = np.arange(-(S - 1), S, dtype=np.int32)          # k - q in [-1023, 1023]
    buckets = _rel_pos_bucket_np(rel)                     # [2047]
    table = rel_emb[buckets, :].astype(np.float32)        # [2047, H]
    pp = np.arange(P)[:, None, None]
    dd = np.arange(NDIAG)[None, :, None]
    cc = np.arange(P)[None, None, :]
    idx = 1023 + (NS - 1 - dd) * P + pp - cc              # [128, NDIAG, 128]
    blocks = np.exp(table[idx])                           # [128, NDIAG, 128, H]
    return np.ascontiguousarray(blocks.transpose(3, 0, 1, 2)).astype(BF16NP)


def _q8pair(w, s):
    """Quantize w*s to fp8 and the residual at the same scale; returns the
    concatenation along axis 0 ([2K, M] from [K, M])."""
    ws = np.asarray(w, np.float32) * s
    qw = np.clip(ws, -240.0, 240.0).astype(FP8NP)
    dw = (ws - qw.astype(np.float32)).astype(np.float32)
    qdw = np.clip(dw, -240.0, 240.0).astype(FP8NP)
    return np.ascontiguousarray(np.concatenate([qw, qdw], axis=0))


# ---- device kernel ---------------------------------------------------------
def build_nc():
    nc = bacc.Bacc(None, target_bir_lowering=False)

    x_d = nc.declare_dram_parameter("x", [S, D], F32, isOutput=False)
    wq_d = nc.declare_dram_parameter("wqc", [2 * D, H * HD], FP8, isOutput=False)
    wk_d = nc.declare_dram_parameter("wkc", [2 * D, H * HD], FP8, isOutput=False)
    wv_d = nc.declare_dram_parameter("wvc", [2 * D, H * HD], FP8, isOutput=False)
    wo_d = nc.declare_dram_parameter("woc", [2 * H * HD, D], FP8, isOutput=False)
    wi_d = nc.declare_dram_parameter("wic", [2 * D, MLP], FP8, isOutput=False)
    wm_d = nc.declare_dram_parameter("wmc", [2 * MLP, D], FP8, isOutput=False)
    bias_d = nc.declare_dram_parameter("biasb", [H, P, NDIAG, P], BF16, isOutput=False)
    out_d = nc.declare_dram_parameter("out", [S, D], F32, isOutput=True)

    wq_t = wq_d.ap().rearrange("(di p) m -> p di m", p=P)   # [128, 16, 1024]
    wk_t = wk_d.ap().rearrange("(di p) m -> p di m", p=P)
    wv_t = wv_d.ap().rearrange("(di p) m -> p di m", p=P)
    wo_t = wo_d.ap().rearrange("(hp p) d -> p hp d", p=P)   # [128, 16, 1024]
    wi_t = wi_d.ap().rearrange("(di p) m -> p di m", p=P)   # [128, 16, 4096]
    wm_t = wm_d.ap().rearrange("(ci p) d -> p ci d", p=P)   # [128, 64, 1024]

    with tile.TileContext(nc) as tc:
        _body(nc, tc, x_d, wq_t, wk_t, wv_t, wo_t, wi_t, wm_t, bias_d, out_d)
    nc.finalize()
    return nc


def _rmsnorm_stats(nc, pools, src_ap, eps_t):
    """Returns rstd32 [P,1]: 32 / sqrt(mean(src^2) + eps) for src [128, D]."""
    sq = pools["sc"].tile([P, D], BF16, tag="sq")
    var = pools["nrm"].tile([P, 1], F32, tag="var")
    nc.scalar.activation(out=sq, in_=src_ap, func=mybir.ActivationFunctionType.Square,
                         accum_out=var)
    sd = pools["nrm"].tile([P, 1], F32, tag="sd")
    nc.scalar.activation(out=sd, in_=var, func=mybir.ActivationFunctionType.Sqrt,
                         bias=eps_t[:, :], scale=1.0 / D)
    rstd = pools["nrm"].tile([P, 1], F32, tag="rstd")
    nc.vector.reciprocal(out=rstd, in_=sd)
    return rstd


def _body(nc, tc, x_d, wq_t, wk_t, wv_t, wo_t, wi_t, wm_t, bias_d, out_d):
    fp = {}  # pools

    def pool(name, bufs, space="SBUF"):
        p = tc.alloc_tile_pool(name=name, bufs=bufs, space=space)
        fp[name] = p
        return p

    AF = mybir.ActivationFunctionType
    ALU = mybir.AluOpType
    DR = mybir.MatmulPerfMode.DoubleRow

    singles = pool("singles", 1)
    ident16 = singles.tile([P, P], BF16)
    make_identity(nc, ident16)
    eps_t = singles.tile([P, 1], F32)
    nc.vector.memset(eps_t, EPS)

    pool("sc", 2)      # [128, D] scratch
    pool("nrm", 8)     # [128, 1] norm scalars

    # stage-4 output reserves right-side space from the start; attnT sits
    # above it on the right stack so both can release after stage 4/7 without
    # LIFO conflicts with the attention-phase left-side pools.
    out1_pool = tc.alloc_tile_pool(name="out1_pool", bufs=1, side="right")
    out1 = out1_pool.tile([P, NS, D], F32)    # x + attn_out, token-major
    attnT_pool = tc.alloc_tile_pool(name="attnT_pool", bufs=1, side="right")
    attnT = attnT_pool.tile([P, H // 2, S], FP8)
    dattnT = attnT_pool.tile([P, H // 2, S], FP8)

    # activations that live through the attention block; q/k in bf16 to keep
    # the attention-phase SBUF low enough that stage-4 weights prefetch into
    # reserved space during attention
    qkv_act = tc.alloc_tile_pool(name="qkv_act", bufs=1)
    qT = qkv_act.tile([P, ND, S], BF16)     # q^T  [hhd, s] (scaled by A_QKV)
    kT = qkv_act.tile([P, ND, S], BF16)     # k^T  [hhd, s]
    v_ext = qkv_act.tile([P, NS, H, HD + 1], BF16)  # [tok, stile, h, hd|1]
    nc.vector.memset(v_ext[:, :, :, HD:HD + 1], ONES_VAL)

    hT_pool = tc.alloc_tile_pool(name="hT_pool", bufs=1)
    hqT = hT_pool.tile([P, ND, S], FP8)
    dhqT = hT_pool.tile([P, ND, S], FP8)

    # stage-2 pools/helpers come first: the v-projection matmuls interleave
    # into the stage-1 loop, and qk m-tiles interleave into attention.
    wqkv = tc.alloc_tile_pool(name="wqkv", bufs=3)
    bigp = tc.alloc_tile_pool(name="bigp", bufs=2, space="PSUM")

    def w_half(w_ap, half):
        w_sb = wqkv.tile([P, 2 * ND, 512], FP8, tag="w")
        nc.sync.dma_start(out=w_sb, in_=w_ap[:, :, half * 512:(half + 1) * 512])
        return w_sb

    def cc_matmul(ps_slice, w_sb, mcols, rcols):
        """12 DoubleRow matmuls accumulating Qw.Qa + Qdw.Qa + Qw.Qda."""
        step = 0
        for grp in range(3):
            for j in range(ND // 2):
                if grp == 0:
                    wsl, rsl = 2 * j, hqT
                elif grp == 1:
                    wsl, rsl = ND + 2 * j, hqT
                else:
                    wsl, rsl = 2 * j, dhqT
                nc.tensor.matmul(
                    ps_slice,
                    w_sb[:, wsl:wsl + 2, mcols[0]:mcols[1]],
                    rsl[:, 2 * j:2 * j + 2, rcols[0]:rcols[1]],
                    start=(step == 0), stop=(step == 11), perf_mode=DR,
                )
                step += 1

    def qk_mtile(dstT, w_sb, mj):
        mc = ((mj % 4) * P, (mj % 4 + 1) * P)
        for sh in range(2):
            ps = bigp.tile([P, S], F32, space="PSUM", tag="lg")
            cc_matmul(ps[:, 0:512], w_sb, mc, (sh * 512, (sh + 1) * 512))
            nc.scalar.copy(out=dstT[:, mj, sh * 512:(sh + 1) * 512],
                           in_=ps[:, 0:512])

    def v_ci(w_sb, half, ci):
        ps = bigp.tile([P, S], F32, space="PSUM", tag="lg")
        step = 0
        for grp in range(3):
            for j in range(ND // 2):
                rsl = dhqT if grp == 2 else hqT
                wsl = ND + 2 * j if grp == 1 else 2 * j
                nc.tensor.matmul(
                    ps[:, 0:512],
                    rsl[:, 2 * j:2 * j + 2, ci * P:(ci + 1) * P],
                    w_sb[:, wsl:wsl + 2, 0:512],
                    start=(step == 0), stop=(step == 11), perf_mode=DR,
                )
                step += 1
        nc.vector.tensor_copy(
            out=v_ext[:, ci, half * 8:half * 8 + 8, 0:HD],
            in_=ps[:, 0:512].rearrange("p (h e) -> p h e", e=HD),
        )

    def drain_tp(dq, ddq, psall, si):
        """fp8 slab column + residual from a batched transpose PSUM tile."""
        nc.vector.tensor_copy(out=dq[:, :, si * P:(si + 1) * P], in_=psall[:, :, :])
        nc.vector.scalar_tensor_tensor(
            out=ddq[:, :, si * P:(si + 1) * P],
            in0=psall[:, :, :], scalar=1.0,
            in1=dq[:, :, si * P:(si + 1) * P],
            op0=ALU.mult, op1=ALU.subtract,
        )

    # ---- stage 1: rmsnorm(x) -> hqT + dhqT; v-projection interleaved -------
    # 8 transposes batch into one 1-bank PSUM tile; a single DVE copy and a
    # single DVE subtract produce the fp8 slab column + its residual one tile
    # behind, and the v m-tile for the drained column runs right after.
    wv0 = wv1 = None
    with tc.tile_pool(name="xs1", bufs=4) as xs1, \
         tc.tile_pool(name="tp1", bufs=3, space="PSUM") as tp1:
        pend = None
        for si in range(NS):
            xt = xs1.tile([P, D], F32, tag="x")
            nc.sync.dma_start(out=xt, in_=x_d.ap()[si * P:(si + 1) * P, :])
            rstd = _rmsnorm_stats(nc, fp, xt[:, :], eps_t)
            hs = fp["sc"].tile([P, D], BF16, tag="h")
            if si == 0:
                for hh in range(2):
                    nc.scalar.activation(
                        out=hs[:, hh * 512:(hh + 1) * 512],
                        in_=xt[:, hh * 512:(hh + 1) * 512], func=AF.Copy,
                        bias=0.0, scale=rstd[:, :])
            else:
                nc.scalar.activation(out=hs, in_=xt[:, :], func=AF.Copy,
                                     bias=0.0, scale=rstd[:, :])
            psall = tp1.tile([P, ND, P], BF16, space="PSUM", tag="tp")
            for di in range(ND):
                nc.tensor.transpose(psall[:, di, :], hs[:, di * P:(di + 1) * P],
                                    ident16[:, :])
            if si == 1:
                wv0 = w_half(wv_t, 0)
            if si == 2:
                wv1 = w_half(wv_t, 1)
            if pend is not None:
                drain_tp(hqT, dhqT, *pend)
                v_ci(wv0, 0, pend[1])
                if pend[1] >= 1:
                    v_ci(wv1, 1, pend[1] - 1)
            pend = (psall, si)
        drain_tp(hqT, dhqT, *pend)
        v_ci(wv0, 0, pend[1])
        for ci in range(6, 8):
            v_ci(wv1, 1, ci)

    # ---- stages 2+3 fused: QKV projections interleaved into attention ------
    # Attention is ACT(exp)-bound; the fp8 DoubleRow qk matmuls run in PE's
    # idle slots between head blocks. Head h only needs q/k slab h//2 and the
    # v half h//8, so emission order interleaves m-tiles ahead of the heads
    # that need them. PSUM budget: bigp 2x[P,S] (4 banks) + au 2x2 = 8 banks.
    aup = tc.alloc_tile_pool(name="aup", bufs=2, space="PSUM")
    biasp = tc.alloc_tile_pool(name="biasp", bufs=2)
    wexpp = tc.alloc_tile_pool(name="wexpp", bufs=5)
    rp = tc.alloc_tile_pool(name="rp", bufs=2)

    # stage-4 weights reserve right-side space now so the wo DMA can land
    # mid-attention instead of waiting for attention SBUF to drain
    wop = tc.alloc_tile_pool(name="wop", bufs=1, side="right")
    wo_sb = wop.tile([P, 2 * (H // 2), D], FP8)

    def head_block(h):
        hb = HD * (h % 2)           # partition base of this head in qT/kT
        hm = h // 2
        bias_sb = biasp.tile([P, NDIAG, P], BF16, tag="bias")
        nc.sync.dma_start(out=bias_sb, in_=bias_d.ap()[h])
        au = aup.tile([HD + 1, S], F32, space="PSUM", tag="au")

        def attn_v(ki, wexp):
            for qh in range(2):
                nc.tensor.matmul(
                    au[:, qh * 512:(qh + 1) * 512],
                    v_ext[:, ki, h, :],
                    wexp[:, qh * 512:(qh + 1) * 512],
                    start=(ki == 0), stop=(ki == NS - 1),
                )

        # software-pipelined two deep: PE's in-order stream runs ki+1/ki+2
        # logits while ACT/DVE produce wexp(ki), keeping the ACT exp chain
        # (the attention bottleneck) saturated.
        pend = []
        for ki in range(NS):
            lg = bigp.tile([P, S], F32, space="PSUM", tag="lg")
            for qh in range(2):
                nc.tensor.matmul(
                    lg[:, qh * 512:(qh + 1) * 512],
                    kT[hb:hb + HD, hm, ki * P:(ki + 1) * P],
                    qT[hb:hb + HD, hm, qh * 512:(qh + 1) * 512],
                    start=True, stop=True,
                )
            # w = exp(l * EXPSC) * exp(bias): the fp8-GEMM descale folds into
            # the exp scale; then an all-bf16 SBUF multiply on DVE (2x mode)
            ex = wexpp.tile([P, S], BF16, tag="ex")
            nc.scalar.activation(out=ex, in_=lg[:, :], func=AF.Exp, scale=EXPSC)
            wexp = wexpp.tile([P, S], BF16, tag="wexp")
            nc.vector.tensor_mul(
                out=wexp[:, :].rearrange("p (c w) -> p c w", w=P),
                in0=ex[:, :].rearrange("p (c w) -> p c w", w=P),
                in1=bias_sb[:, NS - 1 - ki:2 * NS - 1 - ki, :],
            )
            pend.append((ki, wexp))
            if len(pend) > 1:
                attn_v(*pend.pop(0))
        for pe_ in pend:
            attn_v(*pe_)

        # attnT = num/den via reciprocal of the scaled ones-row
        hb2 = HD * (h % 2)
        rden = rp.tile([1, S], BF16, tag="rden")
        with nc.allow_low_precision(reason="denominator broadcast in bf16"):
            nc.vector.reciprocal(out=rden, in_=au[HD:HD + 1, :])
        rbc = rp.tile([HD, S], BF16, tag="rbc")
        nc.gpsimd.partition_broadcast(rbc[:, :], rden[:, :])
        # t is full-height so its slice shares the base partition of the
        # attnT slice (walrus requires equal SBUF base partitions for
        # two-SBUF-input ops like the residual subtract below)
        t = rp.tile([P, S], BF16, tag="t")
        ts_ = t[hb2:hb2 + HD, :]
        nc.vector.tensor_mul(out=ts_, in0=au[0:HD, :], in1=rbc[:, :])
        nc.vector.tensor_copy(out=attnT[hb2:hb2 + HD, h // 2, :], in_=ts_)
        # walrus rejects TensorScalarPtr on Pool, so the residual stays on DVE
        nc.vector.scalar_tensor_tensor(
            out=dattnT[hb2:hb2 + HD, h // 2, :],
            in0=ts_, scalar=1.0, in1=attnT[hb2:hb2 + HD, h // 2, :],
            op0=ALU.mult, op1=ALU.subtract,
        )

    wq0 = w_half(wq_t, 0)
    wk0 = w_half(wk_t, 0)
    qk_mtile(qT, wq0, 0)
    qk_mtile(kT, wk0, 0)
    head_block(0)
    head_block(1)
    qk_mtile(qT, wq0, 1)
    qk_mtile(kT, wk0, 1)
    head_block(2)
    head_block(3)
    qk_mtile(qT, wq0, 2)
    qk_mtile(kT, wk0, 2)
    head_block(4)
    head_block(5)
    qk_mtile(qT, wq0, 3)
    qk_mtile(kT, wk0, 3)
    head_block(6)
    wq1 = w_half(wq_t, 1)
    wk1 = w_half(wk_t, 1)
    head_block(7)
    qk_mtile(qT, wq1, 4)
    qk_mtile(kT, wk1, 4)
    head_block(8)
    nc.sync.dma_start(out=wo_sb, in_=wo_t)
    qk_mtile(qT, wq1, 5)
    qk_mtile(kT, wk1, 5)
    head_block(9)
    head_block(10)
    head_block(11)
    qk_mtile(qT, wq1, 6)
    qk_mtile(kT, wk1, 6)
    head_block(12)
    head_block(13)
    qk_mtile(qT, wq1, 7)
    qk_mtile(kT, wk1, 7)
    head_block(14)
    head_block(15)

    rp.release()
    wexpp.release()
    biasp.release()
    aup.release()
    bigp.release()
    wqkv.release()
    hT_pool.release()
    qkv_act.release()

    # wi chunks prefetch on the left stack before stage-4's x/weight traffic
    wip = tc.alloc_tile_pool(name="wip", bufs=4)

    wi_tiles = {}

    def wi_chunk(e):
        t_ = wip.tile([P, 2 * ND, MLP // 8], FP8, tag="wi")
        nc.sync.dma_start(out=t_, in_=wi_t[:, :, e * 512:(e + 1) * 512])
        wi_tiles[e] = t_

    wi_chunk(0)

    # ---- stage 4: attn @ wo + residual -------------------------------------
    # Two psum groups in flight: the next tile's steps for head slabs 0-5
    # (j<=2) run before this tile's last-head slabs (j=3), so the final
    # head's normalize tail hides behind ~4us of matmuls.
    with tc.tile_pool(name="xs4", bufs=3) as xs4, \
         tc.tile_pool(name="ops", bufs=2, space="PSUM") as ops:

        def s4_steps(si, ps, step, js):
            for dh in range(2):
                for j in js:
                    for grp in range(3):
                        if grp == 0:
                            a_sl, woffs = attnT, 2 * j
                        elif grp == 1:
                            a_sl, woffs = attnT, 8 + 2 * j
                        else:
                            a_sl, woffs = dattnT, 2 * j
                        nc.tensor.matmul(
                            ps[:, dh * 512:(dh + 1) * 512],
                            a_sl[:, 2 * j:2 * j + 2, si * P:(si + 1) * P],
                            wo_sb[:, woffs:woffs + 2, dh * 512:(dh + 1) * 512],
                            start=(step[dh] == 0), stop=(step[dh] == 11),
                            perf_mode=DR,
                        )
                        step[dh] += 1

        def s4_finish(si, ps, step, xt):
            s4_steps(si, ps, step, (3,))
            nc.vector.scalar_tensor_tensor(
                out=out1[:, si, :], in0=ps[:, :], scalar=B_WO, in1=xt[:, :],
                op0=ALU.mult, op1=ALU.add,
            )

        pending = []
        for si in range(NS):
            xt = xs4.tile([P, D], F32, tag="x")
            nc.sync.dma_start(out=xt, in_=x_d.ap()[si * P:(si + 1) * P, :])
            if si == 1:
                wi_chunk(1)
            ps = ops.tile([P, D], F32, tag="wo")
            step = [0, 0]
            s4_steps(si, ps, step, (0, 1, 2))
            pending.append((si, ps, step, xt))
            if len(pending) > 1:
                s4_finish(*pending.pop(0))
        while pending:
            s4_finish(*pending.pop(0))
    wop.release()
    attnT_pool.release()

    # ---- stages 5+6 fused: rmsnorm(out1) -> h2qT/dh2qT, wi GEMM ------------
    # Stage-6 matmul jobs for the first token half interleave into the
    # stage-5 loop as soon as their h2qT columns drain.
    yT_pool = tc.alloc_tile_pool(name="yT_pool", bufs=1)
    yT = yT_pool.tile([P, NM, S], FP8)
    dyT = yT_pool.tile([P, NM, S], FP8)
    # qWm half of womlp stays resident (loaded once, prefetched early); the
    # dWm residual half streams through a small pool in stage 7.
    wmq_pool = tc.alloc_tile_pool(name="wmq_pool", bufs=1)
    wmq = wmq_pool.tile([P, NM, D], FP8)
    h2T_pool = tc.alloc_tile_pool(name="h2T_pool", bufs=1, side="right")
    h2qT = h2T_pool.tile([P, ND, S], FP8)

    with tc.tile_pool(name="tp5", bufs=2, space="PSUM") as tp5, \
         tc.tile_pool(name="psy", bufs=4, space="PSUM") as psy:

        def s6_job(e, mj, sh):
            # wi runs weight-compensated only (8 steps): the h2 activation
            # quantization error costs ~0.004 rel here, within budget, and
            # saves the third matmul pass plus the dh2qT residual tensor.
            m0 = e * 4 + mj
            sl = slice(sh * 512, (sh + 1) * 512)
            ps = psy.tile([P, 512], F32, space="PSUM", tag="y")
            step = 0
            for grp in range(2):
                for j in range(ND // 2):
                    wsl = ND + 2 * j if grp == 1 else 2 * j
                    nc.tensor.matmul(
                        ps[:, :],
                        wi_tiles[e][:, wsl:wsl + 2, mj * P:(mj + 1) * P],
                        h2qT[:, 2 * j:2 * j + 2, sl],
                        start=(step == 0), stop=(step == 7), perf_mode=DR,
                    )
                    step += 1
            nc.scalar.activation(out=yT[:, m0, sl], in_=ps[:, :], func=AF.Relu)
            # dyT = relu(ps) - yT: quantization residual of the relu output
            # (Pool can't read PSUM on HW so this stays on DVE)
            nc.vector.scalar_tensor_tensor(
                out=dyT[:, m0, sl], in0=ps[:, :], scalar=0.0, in1=yT[:, m0, sl],
                op0=ALU.max, op1=ALU.subtract,
            )

        pend = None
        for si in range(NS):
            rstd = _rmsnorm_stats(nc, fp, out1[:, si, :], eps_t)
            h2 = fp["sc"].tile([P, D], BF16, tag="h")
            nc.scalar.activation(out=h2, in_=out1[:, si, :], func=AF.Copy,
                                 bias=0.0, scale=rstd[:, :])
            psall = tp5.tile([P, ND, P], BF16, space="PSUM", tag="tp16")
            for di in range(ND):
                nc.tensor.transpose(psall[:, di, :], h2[:, di * P:(di + 1) * P],
                                    ident16[:, :])
            if pend is not None:
                nc.vector.tensor_copy(
                    out=h2qT[:, :, pend[1] * P:(pend[1] + 1) * P],
                    in_=pend[0][:, :, :])
            pend = (psall, si)
            if si >= 4:
                e = si - 4
                if si in (5, 6):
                    wi_chunk(si - 3)
                for mj in range(4):
                    s6_job(e, mj, 0)
        nc.vector.tensor_copy(
            out=h2qT[:, :, pend[1] * P:(pend[1] + 1) * P], in_=pend[0][:, :, :])
        nc.sync.dma_start(out=wmq, in_=wm_t[:, 0:NM, :])
        for e in range(4):
            if e == 0:
                wi_chunk(4)
            for mj in range(4):
                s6_job(e, mj, 1)
        for e in range(4, 8):
            if e + 1 <= 7:
                wi_chunk(e + 1)
            for mj in range(4):
                s6_job(e, mj, 0)
            for mj in range(4):
                s6_job(e, mj, 1)
    h2T_pool.release()

    # ---- stage 7: out = out1 + y^T.T @ womlp (fp8 DoubleRow, 3-pass) -------
    # dh-major per psum group so the first half's epilogue (scale-add + store)
    # overlaps the second half's matmuls.
    with tc.tile_pool(name="wmdp", bufs=4) as wmdp, \
         tc.tile_pool(name="oop", bufs=2) as oop, \
         tc.tile_pool(name="o2ps", bufs=4, space="PSUM") as o2ps:
        for sg in range(2):
            pss = [o2ps.tile([P, D], F32, tag="o2", name=f"o2_{sg}_{i}") for i in range(4)]
            step = {}
            wmcs = []
            for ch in range(4):
                wmc = wmdp.tile([P, NS, D], FP8, tag="wm")
                nc.sync.dma_start(out=wmc, in_=wm_t[:, NM + ch * 8:NM + (ch + 1) * 8, :])
                wmcs.append(wmc)

            def mm(i4, dh2, rt, rslab, wt, wslab):
                si = sg * 4 + i4
                k = (i4, dh2)
                nc.tensor.matmul(
                    pss[i4][:, dh2 * 512:(dh2 + 1) * 512],
                    rt[:, rslab:rslab + 2, si * P:(si + 1) * P],
                    wt[:, wslab:wslab + 2, dh2 * 512:(dh2 + 1) * 512],
                    start=(step.get(k, 0) == 0), stop=(step.get(k, 0) == 47),
                    perf_mode=DR,
                )
                step[k] = step.get(k, 0) + 1

            for i4 in range(4):
                si = sg * 4 + i4
                for dh2 in range(2):
                    # resident qWm steps first: PE stays dense while dWm lands
                    for j in range(NM // 2):
                        mm(i4, dh2, yT, 2 * j, wmq, 2 * j)
                    for j in range(NM // 2):
                        mm(i4, dh2, dyT, 2 * j, wmq, 2 * j)
                    for ch in range(4):
                        for j in range(4):
                            mm(i4, dh2, yT, ch * 8 + 2 * j, wmcs[ch], 2 * j)
                    sl = slice(dh2 * 512, (dh2 + 1) * 512)
                    oo = oop.tile([P, 512], F32, tag="oo")
                    nc.vector.scalar_tensor_tensor(
                        out=oo[:, :], in0=pss[i4][:, sl], scalar=B_WM,
                        in1=out1[:, si, sl], op0=ALU.mult, op1=ALU.add,
                    )
                    nc.sync.dma_start(
                        out=out_d.ap()[si * P:(si + 1) * P, sl], in_=oo[:, :])

    wmq_pool.release()
    yT_pool.release()
    wip.release()
    out1_pool.release()
    for name in ("nrm", "sc", "singles"):
        fp[name].release()


# ---- host wrapper ----------------------------------------------------------
_NC_CACHE = {}


def _get_nc():
    if "nc" not in _NC_CACHE:
        _NC_CACHE["nc"] = build_nc()
    return _NC_CACHE["nc"]


def _get_exec():
    """Compile once: a sharded PJRT executable over the 8 NeuronCores."""
    if "exec" in _NC_CACHE:
        return _NC_CACHE["exec"]
    import jax
    from jax.sharding import Mesh, PartitionSpec, NamedSharding
    from jax.experimental.shard_map import shard_map
    from concourse.bass2jax import (
        _bass_exec_p, install_neuronx_cc_hook, partition_id_tensor,
    )

    nc = _get_nc()
    install_neuronx_cc_hook()
    pname = nc.partition_id_tensor.name if nc.partition_id_tensor else None
    in_names, out_names, out_avals, zero_outs = [], [], [], []
    for alloc in nc.m.functions[0].allocations:
        if not isinstance(alloc, mybir.MemoryLocationSet):
            continue
        name = alloc.memorylocations[0].name
        if alloc.kind == "ExternalInput":
            if name != pname:
                in_names.append(name)
        elif alloc.kind == "ExternalOutput":
            out_names.append(name)
            shape = tuple(alloc.tensor_shape)
            dtype = mybir.dt.np(alloc.dtype)
            out_avals.append(jax.core.ShapedArray(shape, dtype))
            zero_outs.append(np.zeros(shape, dtype))
    n_params = len(in_names)
    all_in_names = in_names + out_names + ([pname] if pname else [])

    def _body(*args):
        operands = list(args)
        if pname is not None:
            operands.append(partition_id_tensor())
        outs = _bass_exec_p.bind(
            *operands,
            out_avals=tuple(out_avals),
            in_names=tuple(all_in_names),
            out_names=tuple(out_names),
            lowering_input_output_aliases=(),
            sim_require_finite=True,
            sim_require_nnan=True,
            nc=nc,
        )
        return tuple(outs)

    n_outs = len(out_avals)
    devices = jax.devices()[:NCORES]
    mesh = Mesh(np.asarray(devices), ("core",))
    sharded = jax.jit(
        shard_map(_body, mesh=mesh,
                  in_specs=(PartitionSpec("core"),) * (n_params + n_outs),
                  out_specs=(PartitionSpec("core"),) * n_outs,
                  check_rep=False),
        donate_argnums=tuple(range(n_params, n_params + n_outs)),
        keep_unused=True,
    )
    sh = NamedSharding(mesh, PartitionSpec("core"))
    _NC_CACHE["exec"] = (sharded, in_names, out_names, zero_outs, sh)
    return _NC_CACHE["exec"]


def _prep_inputs(x, ln1_scale, wq, wk, wv, wo_attn, ln2_scale, wi, wo_mlp, rel_emb):
    x = np.asarray(x, np.float32)
    ln1 = np.asarray(ln1_scale, np.float32)[:, None]
    ln2 = np.asarray(ln2_scale, np.float32)[:, None]
    shared = {
        "wqc": _q8pair(np.asarray(wq, np.float32) * ln1, SW),
        "wkc": _q8pair(np.asarray(wk, np.float32) * ln1, SW),
        "wvc": _q8pair(np.asarray(wv, np.float32) * ln1, SW),
        "woc": _q8pair(np.asarray(wo_attn, np.float32), SW),
        "wic": _q8pair(np.asarray(wi, np.float32) * ln2, SWI),
        "wmc": _q8pair(np.asarray(wo_mlp, np.float32), SWM),
        "biasb": _bias_blocks(np.asarray(rel_emb, np.float32)),
    }
    in_maps = [dict(shared, x=np.ascontiguousarray(x[b])) for b in range(NCORES)]
    return in_maps


def kernel(x, ln1_scale, wq, wk, wv, wo_attn, ln2_scale, wi, wo_mlp, rel_emb):
    import jax
    in_maps = _prep_inputs(x, ln1_scale, wq, wk, wv, wo_attn, ln2_scale,
                           wi, wo_mlp, rel_emb)
    sharded, in_names, out_names, zero_outs, sh = _get_exec()
    concat_in = [
        jax.device_put(
            np.concatenate([in_maps[c][n] for c in range(NCORES)], axis=0), sh)
        for n in in_names
    ]
    czero = [
        jax.device_put(np.zeros((NCORES * z.shape[0], *z.shape[1:]), z.dtype), sh)
        for z in zero_outs
    ]
    outs = sharded(*concat_in, *czero)
    oidx = out_names.index("out")
    full = np.asarray(outs[oidx]).reshape(NCORES, S, D)
    return full.astype(np.float32)
